# revision 2
# baseline (speedup 1.0000x reference)
"""Trainium2 Bass kernel: GNN message passing (2x LEConv) + edge-scoring MLP
+ per-graph top-k, graph-data-parallel over 8 NeuronCores.

Device (per core = 16 graphs, 4096 nodes, 65536 edges):
  NEFF-A: both LEConv layers. Aggregation uses the per-graph dense adjacency
          W_g (edge-list -> dense format conversion done host-side):
          segment_sum(ew*(a[row]-b[col]), col) == W_g^T-contract(A1) - deg*B1.
          All matmuls/elementwise/relu on TensorE/VectorE/ScalarE.
  NEFF-B: full edge-scoring MLP for all edges:
          score = relu(cat(h2[row],h2[col]) @ Wm1^T + bm1) @ Wm2^T + bm2.

Host does sharding, index-only layout transforms (including the node-row
pre-gather by edge endpoints between the NEFFs), and the final top-k /
relabel index assembly on device-computed scores.
"""

import numpy as np

G, NPG, EPG = 128, 256, 4096
N, E = G * NPG, G * EPG
IN = H = 64
K = EPG // 2
NCORES = 8
GPC = G // NCORES
NPS = GPC * NPG            # 4096 nodes / shard
EPS = GPC * EPG            # 65536 edges / shard

_cache = {}


def _patch_tile():
    import concourse.mybir as mybir
    import concourse.tile as tile_mod
    from concourse.vector_clock import ScopedClock

    if getattr(tile_mod.TileContext, "_ant_patched", False):
        return

    def _drain_and_barrier(self, tick_clock, wait_clock):
        drain_binst = self.nc.sync.drain()
        wait_clock.add_sem_waits(
            drain_binst.ins, ScopedClock({None: tick_clock.global_clock})
        )
        mi = drain_binst.ins
        si = mi.sync_info
        if si is not None and si.on_wait is not None and len(si.on_wait) > 1:
            waits = list(si.on_wait)
            si.on_wait = [waits[0]]
            for w in waits[1:]:
                d2 = self.nc.sync.drain().ins
                if d2.sync_info is None:
                    d2.sync_info = mybir.SyncInfo(on_wait=[w], on_update=[])
                else:
                    d2.sync_info.on_wait = [w]
        self.nc.all_engine_barrier()
        popped = self.nc._tile_sem_poison_stack.pop()
        assert popped is self._sem_poison
        self.nc.clear_and_free_semaphores(list(self.sems.allocated().values()))
        self.nc.all_engine_barrier()

    tile_mod.TileContext._drain_and_barrier = _drain_and_barrier
    tile_mod.TileContext._ant_patched = True


def _split_sync_waits(nc):
    """walrus here allows one sync-wait per instruction; hoist extras onto
    injected same-engine drains."""
    import concourse.mybir as mybir

    for f in nc.m.functions:
        for bb in f.blocks:
            out = []
            changed = False
            for inst in bb.instructions:
                si = inst.sync_info
                waits = list(si.on_wait) if (si is not None and si.on_wait) else []
                if len(waits) > 1:
                    changed = True
                    for w in waits[:-1]:
                        d = mybir.InstDrain(name=f"WF-{nc.next_id()}", ins=[], outs=[])
                        d.engine = inst.engine
                        d.sync_info = mybir.SyncInfo(on_wait=[w], on_update=[])
                        out.append(d)
                    si.on_wait = [waits[-1]]
                out.append(inst)
            if changed:
                bb.instructions = out


def _build_neff_a():
    import concourse.bass as bass
    import concourse.mybir as mybir
    from concourse.tile import TileContext
    from concourse.masks import make_identity

    _patch_tile()
    f32 = mybir.dt.float32
    nc = bass.Bass()
    xT_e = nc.declare_dram_parameter("xT", [64, NPS], f32, isOutput=False)
    # wg layout host-prepped: [128, GPC*512]; wg[p, g*512 + kp*256 + d]
    #   = W_g[g, s = kp*128 + p, d]  (s on partitions, split in 2 k-passes)
    wg_e = nc.declare_dram_parameter("wg", [128, GPC * 512], f32, isOutput=False)
    w1_e = nc.declare_dram_parameter("w1", [64, 192], f32, isOutput=False)
    w2_e = nc.declare_dram_parameter("w2", [64, 192], f32, isOutput=False)
    b1_e = nc.declare_dram_parameter("b1", [128, 192], f32, isOutput=False)
    b2_e = nc.declare_dram_parameter("b2", [128, 192], f32, isOutput=False)
    h2_e = nc.declare_dram_parameter("h2", [NPS, 64], f32, isOutput=True)

    with TileContext(nc) as tc:
        with (
            tc.tile_pool(name="const", bufs=1) as cpool,
            tc.tile_pool(name="abcp", bufs=1) as apool,
            tc.tile_pool(name="work", bufs=4) as wpool,
            tc.tile_pool(name="ps_abc", bufs=2, space="PSUM") as p_abc,
            tc.tile_pool(name="ps_agg", bufs=2, space="PSUM") as p_agg,
            tc.tile_pool(name="ps_tp", bufs=2, space="PSUM") as p_tp,
        ):
            xT = cpool.tile([64, NPS], f32)
            nc.sync.dma_start(out=xT[:], in_=xT_e[:])
            wg = cpool.tile([128, GPC * 512], f32)
            nc.sync.dma_start(out=wg[:], in_=wg_e[:])
            w1 = cpool.tile([64, 192], f32)
            nc.sync.dma_start(out=w1[:], in_=w1_e[:])
            w2 = cpool.tile([64, 192], f32)
            nc.sync.dma_start(out=w2[:], in_=w2_e[:])
            b1 = cpool.tile([128, 192], f32)
            nc.sync.dma_start(out=b1[:], in_=b1_e[:])
            b2 = cpool.tile([128, 192], f32)
            nc.sync.dma_start(out=b2[:], in_=b2_e[:])
            ident = cpool.tile([128, 128], f32)
            make_identity(nc, ident[:])
            ones = cpool.tile([128, 1], f32)
            nc.vector.memset(ones[:], 1.0)

            def layer(src_fT, wcat, brep, relu, out_cb, tagp):
                # 1) ABC[c] = node-major [128, 192] (A|B|C) for 32 node-chunks
                abc_tiles = []
                for c in range(32):
                    ps = p_abc.tile([128, 192], f32, tag="abc_ps")
                    nc.tensor.matmul(ps[:], src_fT(c), wcat[:],
                                     start=True, stop=True)
                    abc = apool.tile([128, 192], f32, tag=f"{tagp}abc{c}")
                    nc.vector.tensor_add(abc[:], ps[:], brep[:])
                    abc_tiles.append(abc)
                # 2) per (graph, d-half): agg (+ deg via ones column)
                for g in range(GPC):
                    for hh in range(2):
                        ps = p_agg.tile([128, 65], f32, tag="agg_ps")
                        for kp in range(2):
                            lhsT = wg[:, g * 512 + kp * 256 + hh * 128:
                                      g * 512 + kp * 256 + hh * 128 + 128]
                            rhs = abc_tiles[g * 2 + kp]
                            nc.tensor.matmul(ps[:, 0:64], lhsT, rhs[:, 0:64],
                                             start=(kp == 0), stop=(kp == 1))
                        for kp in range(2):
                            lhsT = wg[:, g * 512 + kp * 256 + hh * 128:
                                      g * 512 + kp * 256 + hh * 128 + 128]
                            nc.tensor.matmul(ps[:, 64:65], lhsT, ones[:],
                                             start=(kp == 0), stop=(kp == 1))
                        c = g * 2 + hh
                        abc = abc_tiles[c]
                        t1 = wpool.tile([128, 64], f32, tag="t1")
                        nc.vector.tensor_scalar_mul(t1[:], abc[:, 64:128],
                                                    ps[:, 64:65])
                        t2 = wpool.tile([128, 64], f32, tag="t2")
                        nc.vector.tensor_sub(t2[:], ps[:, 0:64], t1[:])
                        hout = wpool.tile([128, 64], f32, tag="hout")
                        nc.vector.tensor_add(hout[:], t2[:], abc[:, 128:192])
                        if relu:
                            nc.scalar.activation(
                                hout[:], hout[:],
                                mybir.ActivationFunctionType.Relu)
                        out_cb(c, hout)

            h1T_tiles = [None] * 32

            def l1_out(c, hout):
                tp = p_tp.tile([64, 128], f32, tag="h1t_ps")
                nc.tensor.transpose(tp[:], hout[:], ident[:])
                h1T = apool.tile([64, 128], f32, tag=f"h1T{c}")
                nc.vector.tensor_copy(h1T[:], tp[:])
                h1T_tiles[c] = h1T

            layer(lambda c: xT[:, c * 128:(c + 1) * 128], w1, b1, True,
                  l1_out, "L1")

            def l2_out(c, hout):
                nc.sync.dma_start(out=h2_e[c * 128:(c + 1) * 128, :], in_=hout[:])

            layer(lambda c: h1T_tiles[c][:], w2, b2, False, l2_out, "L2")

    _split_sync_waits(nc)
    return nc


def _build_neff_b():
    import concourse.bass as bass
    import concourse.mybir as mybir
    from concourse.tile import TileContext

    _patch_tile()
    f32 = mybir.dt.float32
    nc = bass.Bass()
    er_e = nc.declare_dram_parameter("erT", [128, EPS], f32, isOutput=False)
    wm1_e = nc.declare_dram_parameter("wm1", [128, 256], f32, isOutput=False)
    bm1_e = nc.declare_dram_parameter("bm1h", [128, 2], f32, isOutput=False)
    wm2_e = nc.declare_dram_parameter("wm2", [128, 2], f32, isOutput=False)
    bm2_e = nc.declare_dram_parameter("bm2v", [1, 1], f32, isOutput=False)
    sc_e = nc.declare_dram_parameter("sc", [1, EPS], f32, isOutput=True)

    CH = 512
    with TileContext(nc) as tc:
        with (
            tc.tile_pool(name="const", bufs=1) as cpool,
            tc.tile_pool(name="work", bufs=4) as wpool,
            tc.tile_pool(name="ps_z", bufs=4, space="PSUM") as p_z,
            tc.tile_pool(name="ps_s", bufs=4, space="PSUM") as p_s,
        ):
            wm1 = cpool.tile([128, 256], f32)
            nc.sync.dma_start(out=wm1[:], in_=wm1_e[:])
            bm1h = cpool.tile([128, 2], f32)
            nc.sync.dma_start(out=bm1h[:], in_=bm1_e[:])
            wm2 = cpool.tile([128, 2], f32)
            nc.sync.dma_start(out=wm2[:], in_=wm2_e[:])
            bm2 = cpool.tile([1, 1], f32)
            nc.sync.dma_start(out=bm2[:], in_=bm2_e[:])

            for c0 in range(0, EPS, CH):
                er = wpool.tile([128, CH], f32, tag="er")
                nc.sync.dma_start(out=er[:], in_=er_e[:, c0:c0 + CH])
                sps = p_s.tile([1, CH], f32, tag="sps")
                for hh in range(2):
                    zps = p_z.tile([128, CH], f32, tag="zps")
                    nc.tensor.matmul(zps[:], wm1[:, hh * 128:(hh + 1) * 128],
                                     er[:], start=True, stop=True)
                    y = wpool.tile([128, CH], f32, tag="y")
                    nc.scalar.activation(y[:], zps[:],
                                         mybir.ActivationFunctionType.Relu,
                                         bias=bm1h[:, hh:hh + 1])
                    nc.tensor.matmul(sps[:], wm2[:, hh:hh + 1], y[:],
                                     start=(hh == 0), stop=(hh == 1))
                sc = wpool.tile([1, CH], f32, tag="sc")
                nc.vector.tensor_scalar_add(sc[:], sps[:], bm2[:])
                nc.sync.dma_start(out=sc_e[:, c0:c0 + CH], in_=sc[:])

    _split_sync_waits(nc)
    return nc


def _get_neffs():
    if "a" not in _cache:
        _cache["a"] = _build_neff_a()
        _cache["b"] = _build_neff_b()
    return _cache["a"], _cache["b"]


def kernel(x, edge_attr, W11, b11, W12, W13, b13, W21, b21, W22, W23, b23,
           Wm1, bm1, Wm2, bm2, edge_index, batch):
    from concourse.bass_utils import run_bass_kernel_spmd

    x = np.asarray(x, np.float32)
    edge_attr = np.asarray(edge_attr, np.float32)
    ei = np.asarray(edge_index)
    idt = ei.dtype
    nc_a, nc_b = _get_neffs()

    # ---- host prep: format conversions (index-driven layout) ----
    wg = np.zeros((G, NPG, NPG), np.float32)
    np.add.at(wg, (ei[0] // NPG, ei[0] % NPG, ei[1] % NPG),
              edge_attr.astype(np.float32))
    wcat1 = np.ascontiguousarray(
        np.concatenate([W11.T, W12.T, W13.T], axis=1), np.float32)
    wcat2 = np.ascontiguousarray(
        np.concatenate([W21.T, W22.T, W23.T], axis=1), np.float32)
    z64 = np.zeros(64, np.float32)
    b1rep = np.tile(np.concatenate([b11, z64, b13])[None, :], (128, 1)).astype(
        np.float32)
    b2rep = np.tile(np.concatenate([b21, z64, b23])[None, :], (128, 1)).astype(
        np.float32)

    in_maps_a = []
    for c in range(NCORES):
        xs = x[c * NPS:(c + 1) * NPS]
        # wg device layout [128, GPC*512]: [p, g*512 + kp*256 + d]
        wgs = wg[c * GPC:(c + 1) * GPC]                # [GPC, 256, 256]
        wgl = np.ascontiguousarray(
            wgs.reshape(GPC, 2, 128, 256).transpose(2, 0, 1, 3).reshape(
                128, GPC * 512))
        in_maps_a.append({
            "xT": np.ascontiguousarray(xs.T),
            "wg": wgl,
            "w1": wcat1, "w2": wcat2, "b1": b1rep, "b2": b2rep,
        })
    res_a = run_bass_kernel_spmd(nc_a, in_maps_a, list(range(NCORES)))
    h2 = np.concatenate([res_a.results[c]["h2"] for c in range(NCORES)], axis=0)

    # ---- index-only inter-phase gather ----
    wm1T = np.ascontiguousarray(np.asarray(Wm1, np.float32).T)          # [128,256]
    bm1h = np.ascontiguousarray(
        np.asarray(bm1, np.float32).reshape(2, 128).T)                   # [128,2]
    wm2h = np.ascontiguousarray(
        np.asarray(Wm2, np.float32).reshape(2, 128).T)                   # [128,2]
    bm2v = np.full((1, 1), np.asarray(bm2).reshape(-1)[0], np.float32)
    in_maps_b = []
    for c in range(NCORES):
        e0 = c * EPS
        r = ei[0, e0:e0 + EPS]
        cl = ei[1, e0:e0 + EPS]
        erT = np.empty((128, EPS), np.float32)
        erT[0:64, :] = h2[r].T
        erT[64:128, :] = h2[cl].T
        in_maps_b.append({"erT": erT, "wm1": wm1T, "bm1h": bm1h,
                          "wm2": wm2h, "bm2v": bm2v})
    res_b = run_bass_kernel_spmd(nc_b, in_maps_b, list(range(NCORES)))
    score = np.concatenate(
        [res_b.results[c]["sc"].reshape(-1) for c in range(NCORES)])

    # ---- host index assembly (mirrors reference) ----
    s = score.reshape(G, EPG)
    order = np.argsort(-s, axis=1, kind="stable")
    keep, drop = order[:, :K], order[:, K:]
    causal_w = np.take_along_axis(s, keep, axis=1).reshape(-1)
    conf_w = -np.take_along_axis(s, drop, axis=1).reshape(-1)
    eoffs = (np.arange(G) * EPG)[:, None]
    kg = (keep + eoffs).reshape(-1)
    dg = (drop + eoffs).reshape(-1)
    causal_ei = ei[:, kg]
    conf_ei = ei[:, dg]

    def relabel(e):
        present = np.zeros((N,), np.int32)
        present[e.reshape(-1)] = 1
        nid = np.where(present == 1, np.cumsum(present) - 1, -1).astype(np.int32)
        return nid[e].astype(idt), present.astype(idt)

    causal_rel, causal_mask = relabel(causal_ei)
    conf_rel, conf_mask = relabel(conf_ei)
    out = np.concatenate([score, causal_w, conf_w]).astype(np.float32)
    return out, causal_rel, conf_rel, causal_mask, conf_mask


# revision 5
# speedup vs baseline: 1.1008x; 1.1008x over previous
"""Trainium2 Bass kernel: GNN message passing (2x LEConv) + edge-scoring MLP
+ per-graph top-k, graph-data-parallel over 8 NeuronCores.

Device (per core = 16 graphs, 4096 nodes, 65536 edges):
  NEFF-A: both LEConv layers. Aggregation uses the per-graph dense adjacency
          W_g (edge-list -> dense format conversion done host-side):
          segment_sum(ew*(a[row]-b[col]), col) == W_g^T-contract(A1) - deg*B1.
          All matmuls/elementwise/relu on TensorE/VectorE/ScalarE.
  NEFF-B: full edge-scoring MLP for all edges:
          score = relu(cat(h2[row],h2[col]) @ Wm1^T + bm1) @ Wm2^T + bm2.

Host does sharding, index-only layout transforms (including the node-row
pre-gather by edge endpoints between the NEFFs), and the final top-k /
relabel index assembly on device-computed scores.
"""

import numpy as np

G, NPG, EPG = 128, 256, 4096
N, E = G * NPG, G * EPG
IN = H = 64
K = EPG // 2
NCORES = 8
GPC = G // NCORES
NPS = GPC * NPG            # 4096 nodes / shard
EPS = GPC * EPG            # 65536 edges / shard

_cache = {}


def _patch_tile():
    import concourse.mybir as mybir
    import concourse.tile as tile_mod
    from concourse.vector_clock import ScopedClock

    if getattr(tile_mod.TileContext, "_ant_patched", False):
        return

    def _drain_and_barrier(self, tick_clock, wait_clock):
        drain_binst = self.nc.sync.drain()
        wait_clock.add_sem_waits(
            drain_binst.ins, ScopedClock({None: tick_clock.global_clock})
        )
        mi = drain_binst.ins
        si = mi.sync_info
        if si is not None and si.on_wait is not None and len(si.on_wait) > 1:
            waits = list(si.on_wait)
            si.on_wait = [waits[0]]
            for w in waits[1:]:
                d2 = self.nc.sync.drain().ins
                if d2.sync_info is None:
                    d2.sync_info = mybir.SyncInfo(on_wait=[w], on_update=[])
                else:
                    d2.sync_info.on_wait = [w]
        self.nc.all_engine_barrier()
        popped = self.nc._tile_sem_poison_stack.pop()
        assert popped is self._sem_poison
        self.nc.clear_and_free_semaphores(list(self.sems.allocated().values()))
        self.nc.all_engine_barrier()

    tile_mod.TileContext._drain_and_barrier = _drain_and_barrier
    tile_mod.TileContext._ant_patched = True


def _split_sync_waits(nc):
    """walrus here allows one sync-wait per instruction; hoist extras onto
    injected same-engine drains."""
    import concourse.mybir as mybir

    for f in nc.m.functions:
        for bb in f.blocks:
            out = []
            changed = False
            for inst in bb.instructions:
                si = inst.sync_info
                waits = list(si.on_wait) if (si is not None and si.on_wait) else []
                if len(waits) > 1:
                    changed = True
                    for w in waits[:-1]:
                        d = mybir.InstDrain(name=f"WF-{nc.next_id()}", ins=[], outs=[])
                        d.engine = inst.engine
                        d.sync_info = mybir.SyncInfo(on_wait=[w], on_update=[])
                        out.append(d)
                    si.on_wait = [waits[-1]]
                out.append(inst)
            if changed:
                bb.instructions = out


def _build_neff_a():
    import concourse.bass as bass
    import concourse.mybir as mybir
    from concourse.tile import TileContext
    from concourse.masks import make_identity

    _patch_tile()
    f32 = mybir.dt.float32
    nc = bass.Bass()
    xT_e = nc.declare_dram_parameter("xT", [64, NPS], f32, isOutput=False)
    # wg layout host-prepped: [128, GPC*512]; wg[p, g*512 + kp*256 + d]
    #   = W_g[g, s = kp*128 + p, d]  (s on partitions, split in 2 k-passes)
    wg_e = nc.declare_dram_parameter("wg", [128, GPC * 512], f32, isOutput=False)
    w1_e = nc.declare_dram_parameter("w1", [64, 192], f32, isOutput=False)
    w2_e = nc.declare_dram_parameter("w2", [64, 192], f32, isOutput=False)
    b1_e = nc.declare_dram_parameter("b1", [128, 192], f32, isOutput=False)
    b2_e = nc.declare_dram_parameter("b2", [128, 192], f32, isOutput=False)
    h2_e = nc.declare_dram_parameter("h2", [NPS, 64], f32, isOutput=True)

    with TileContext(nc) as tc:
        with (
            tc.tile_pool(name="const", bufs=1) as cpool,
            tc.tile_pool(name="abcp", bufs=1) as apool,
            tc.tile_pool(name="work", bufs=4) as wpool,
            tc.tile_pool(name="ps_abc", bufs=2, space="PSUM") as p_abc,
            tc.tile_pool(name="ps_agg", bufs=2, space="PSUM") as p_agg,
            tc.tile_pool(name="ps_tp", bufs=2, space="PSUM") as p_tp,
        ):
            xT = cpool.tile([64, NPS], f32)
            nc.sync.dma_start(out=xT[:], in_=xT_e[:])
            wg = cpool.tile([128, GPC * 512], f32)
            nc.sync.dma_start(out=wg[:], in_=wg_e[:])
            w1 = cpool.tile([64, 192], f32)
            nc.sync.dma_start(out=w1[:], in_=w1_e[:])
            w2 = cpool.tile([64, 192], f32)
            nc.sync.dma_start(out=w2[:], in_=w2_e[:])
            b1 = cpool.tile([128, 192], f32)
            nc.sync.dma_start(out=b1[:], in_=b1_e[:])
            b2 = cpool.tile([128, 192], f32)
            nc.sync.dma_start(out=b2[:], in_=b2_e[:])
            ident = cpool.tile([128, 128], f32)
            make_identity(nc, ident[:])
            ones = cpool.tile([128, 1], f32)
            nc.vector.memset(ones[:], 1.0)

            def layer(src_fT, wcat, brep, relu, out_cb, tagp):
                # 1) ABC[c] = node-major [128, 192] (A|B|C) for 32 node-chunks
                abc_tiles = []
                for c in range(32):
                    ps = p_abc.tile([128, 192], f32, tag="abc_ps")
                    nc.tensor.matmul(ps[:], src_fT(c), wcat[:],
                                     start=True, stop=True)
                    abc = apool.tile([128, 192], f32, tag=f"{tagp}abc{c}")
                    nc.vector.tensor_add(abc[:], ps[:], brep[:])
                    abc_tiles.append(abc)
                # 2) per (graph, d-half): agg (+ deg via ones column)
                for g in range(GPC):
                    for hh in range(2):
                        ps = p_agg.tile([128, 65], f32, tag="agg_ps")
                        for kp in range(2):
                            lhsT = wg[:, g * 512 + kp * 256 + hh * 128:
                                      g * 512 + kp * 256 + hh * 128 + 128]
                            rhs = abc_tiles[g * 2 + kp]
                            nc.tensor.matmul(ps[:, 0:64], lhsT, rhs[:, 0:64],
                                             start=(kp == 0), stop=(kp == 1))
                        for kp in range(2):
                            lhsT = wg[:, g * 512 + kp * 256 + hh * 128:
                                      g * 512 + kp * 256 + hh * 128 + 128]
                            nc.tensor.matmul(ps[:, 64:65], lhsT, ones[:],
                                             start=(kp == 0), stop=(kp == 1))
                        c = g * 2 + hh
                        abc = abc_tiles[c]
                        t1 = wpool.tile([128, 64], f32, tag="t1")
                        nc.vector.tensor_scalar_mul(t1[:], abc[:, 64:128],
                                                    ps[:, 64:65])
                        t2 = wpool.tile([128, 64], f32, tag="t2")
                        nc.vector.tensor_sub(t2[:], ps[:, 0:64], t1[:])
                        hout = wpool.tile([128, 64], f32, tag="hout")
                        nc.vector.tensor_add(hout[:], t2[:], abc[:, 128:192])
                        if relu:
                            nc.scalar.activation(
                                hout[:], hout[:],
                                mybir.ActivationFunctionType.Relu)
                        out_cb(c, hout)

            h1T_tiles = [None] * 32

            def l1_out(c, hout):
                tp = p_tp.tile([64, 128], f32, tag="h1t_ps")
                nc.tensor.transpose(tp[:], hout[:], ident[:])
                h1T = apool.tile([64, 128], f32, tag=f"h1T{c}")
                nc.vector.tensor_copy(h1T[:], tp[:])
                h1T_tiles[c] = h1T

            layer(lambda c: xT[:, c * 128:(c + 1) * 128], w1, b1, True,
                  l1_out, "L1")

            def l2_out(c, hout):
                nc.sync.dma_start(out=h2_e[c * 128:(c + 1) * 128, :], in_=hout[:])

            layer(lambda c: h1T_tiles[c][:], w2, b2, False, l2_out, "L2")

    _split_sync_waits(nc)
    return nc


def _build_neff_b():
    import concourse.bass as bass
    import concourse.mybir as mybir
    from concourse.tile import TileContext

    _patch_tile()
    f32 = mybir.dt.float32
    nc = bass.Bass()
    er_e = nc.declare_dram_parameter("erT", [128, EPS], f32, isOutput=False)
    wm1_e = nc.declare_dram_parameter("wm1", [128, 256], f32, isOutput=False)
    bm1_e = nc.declare_dram_parameter("bm1h", [128, 2], f32, isOutput=False)
    wm2_e = nc.declare_dram_parameter("wm2", [128, 2], f32, isOutput=False)
    bm2_e = nc.declare_dram_parameter("bm2v", [1, 1], f32, isOutput=False)
    sc_e = nc.declare_dram_parameter("sc", [1, EPS], f32, isOutput=True)

    CH = 512
    with TileContext(nc) as tc:
        with (
            tc.tile_pool(name="const", bufs=1) as cpool,
            tc.tile_pool(name="work", bufs=4) as wpool,
            tc.tile_pool(name="ps_z", bufs=4, space="PSUM") as p_z,
            tc.tile_pool(name="ps_s", bufs=4, space="PSUM") as p_s,
        ):
            wm1 = cpool.tile([128, 256], f32)
            nc.sync.dma_start(out=wm1[:], in_=wm1_e[:])
            bm1h = cpool.tile([128, 2], f32)
            nc.sync.dma_start(out=bm1h[:], in_=bm1_e[:])
            wm2 = cpool.tile([128, 2], f32)
            nc.sync.dma_start(out=wm2[:], in_=wm2_e[:])
            bm2 = cpool.tile([1, 1], f32)
            nc.sync.dma_start(out=bm2[:], in_=bm2_e[:])

            for c0 in range(0, EPS, CH):
                er = wpool.tile([128, CH], f32, tag="er")
                nc.sync.dma_start(out=er[:], in_=er_e[:, c0:c0 + CH])
                sps = p_s.tile([1, CH], f32, tag="sps")
                for hh in range(2):
                    zps = p_z.tile([128, CH], f32, tag="zps")
                    nc.tensor.matmul(zps[:], wm1[:, hh * 128:(hh + 1) * 128],
                                     er[:], start=True, stop=True)
                    y = wpool.tile([128, CH], f32, tag="y")
                    nc.scalar.activation(y[:], zps[:],
                                         mybir.ActivationFunctionType.Relu,
                                         bias=bm1h[:, hh:hh + 1])
                    nc.tensor.matmul(sps[:], wm2[:, hh:hh + 1], y[:],
                                     start=(hh == 0), stop=(hh == 1))
                sc = wpool.tile([1, CH], f32, tag="sc")
                nc.vector.tensor_scalar_add(sc[:], sps[:], bm2[:])
                nc.sync.dma_start(out=sc_e[:, c0:c0 + CH], in_=sc[:])

    _split_sync_waits(nc)
    return nc


def _get_neffs():
    if "a" not in _cache:
        _cache["a"] = _build_neff_a()
        _cache["b"] = _build_neff_b()
    return _cache["a"], _cache["b"]


last_device_wall_s = None


def kernel(x, edge_attr, W11, b11, W12, W13, b13, W21, b21, W22, W23, b23,
           Wm1, bm1, Wm2, bm2, edge_index, batch):
    import time as _time

    from concourse.bass_utils import run_bass_kernel_spmd

    global last_device_wall_s

    x = np.asarray(x, np.float32)
    edge_attr = np.asarray(edge_attr, np.float32)
    ei = np.asarray(edge_index)
    idt = ei.dtype
    nc_a, nc_b = _get_neffs()

    # ---- host prep: format conversions (index-driven layout) ----
    wg = np.zeros((G, NPG, NPG), np.float32)
    np.add.at(wg, (ei[0] // NPG, ei[0] % NPG, ei[1] % NPG),
              edge_attr.astype(np.float32))
    wcat1 = np.ascontiguousarray(
        np.concatenate([W11.T, W12.T, W13.T], axis=1), np.float32)
    wcat2 = np.ascontiguousarray(
        np.concatenate([W21.T, W22.T, W23.T], axis=1), np.float32)
    z64 = np.zeros(64, np.float32)
    b1rep = np.tile(np.concatenate([b11, z64, b13])[None, :], (128, 1)).astype(
        np.float32)
    b2rep = np.tile(np.concatenate([b21, z64, b23])[None, :], (128, 1)).astype(
        np.float32)

    in_maps_a = []
    for c in range(NCORES):
        xs = x[c * NPS:(c + 1) * NPS]
        # wg device layout [128, GPC*512]: [p, g*512 + kp*256 + d]
        wgs = wg[c * GPC:(c + 1) * GPC]                # [GPC, 256, 256]
        wgl = np.ascontiguousarray(
            wgs.reshape(GPC, 2, 128, 256).transpose(2, 0, 1, 3).reshape(
                128, GPC * 512))
        in_maps_a.append({
            "xT": np.ascontiguousarray(xs.T),
            "wg": wgl,
            "w1": wcat1, "w2": wcat2, "b1": b1rep, "b2": b2rep,
        })
    _t0 = _time.time()
    res_a = run_bass_kernel_spmd(nc_a, in_maps_a, list(range(NCORES)))
    _dev_a = _time.time() - _t0
    h2 = np.concatenate([res_a.results[c]["h2"] for c in range(NCORES)], axis=0)

    # ---- index-only inter-phase gather ----
    wm1T = np.ascontiguousarray(np.asarray(Wm1, np.float32).T)          # [128,256]
    bm1h = np.ascontiguousarray(
        np.asarray(bm1, np.float32).reshape(2, 128).T)                   # [128,2]
    wm2h = np.ascontiguousarray(
        np.asarray(Wm2, np.float32).reshape(2, 128).T)                   # [128,2]
    bm2v = np.full((1, 1), np.asarray(bm2).reshape(-1)[0], np.float32)
    in_maps_b = []
    for c in range(NCORES):
        e0 = c * EPS
        r = ei[0, e0:e0 + EPS]
        cl = ei[1, e0:e0 + EPS]
        erT = np.empty((128, EPS), np.float32)
        erT[0:64, :] = h2[r].T
        erT[64:128, :] = h2[cl].T
        in_maps_b.append({"erT": erT, "wm1": wm1T, "bm1h": bm1h,
                          "wm2": wm2h, "bm2v": bm2v})
    _t0 = _time.time()
    res_b = run_bass_kernel_spmd(nc_b, in_maps_b, list(range(NCORES)))
    _dev_b = _time.time() - _t0
    last_device_wall_s = _dev_a + _dev_b
    score = np.concatenate(
        [res_b.results[c]["sc"].reshape(-1) for c in range(NCORES)])

    # ---- host index assembly (mirrors reference) ----
    s = score.reshape(G, EPG)
    order = np.argsort(-s, axis=1, kind="stable")
    keep, drop = order[:, :K], order[:, K:]
    causal_w = np.take_along_axis(s, keep, axis=1).reshape(-1)
    conf_w = -np.take_along_axis(s, drop, axis=1).reshape(-1)
    eoffs = (np.arange(G) * EPG)[:, None]
    kg = (keep + eoffs).reshape(-1)
    dg = (drop + eoffs).reshape(-1)
    causal_ei = ei[:, kg]
    conf_ei = ei[:, dg]

    def relabel(e):
        present = np.zeros((N,), np.int32)
        present[e.reshape(-1)] = 1
        nid = np.where(present == 1, np.cumsum(present) - 1, -1).astype(np.int32)
        return nid[e].astype(idt), present.astype(idt)

    causal_rel, causal_mask = relabel(causal_ei)
    conf_rel, conf_mask = relabel(conf_ei)
    out = np.concatenate([score, causal_w, conf_w]).astype(np.float32)
    return out, causal_rel, conf_rel, causal_mask, conf_mask


# revision 8
# speedup vs baseline: 5.6894x; 5.1683x over previous
"""Trainium2 Bass kernel: GNN message passing (2x LEConv) + edge-scoring MLP
+ per-graph top-k, graph-data-parallel over 8 NeuronCores.

Device (per core = 16 graphs, 4096 nodes, 65536 edges):
  NEFF-A: both LEConv layers. Aggregation uses the per-graph dense adjacency
          W_g (edge-list -> dense format conversion done host-side):
          segment_sum(ew*(a[row]-b[col]), col) == W_g^T-contract(A1) - deg*B1.
          All matmuls/elementwise/relu on TensorE/VectorE/ScalarE.
  NEFF-B: full edge-scoring MLP for all edges:
          score = relu(cat(h2[row],h2[col]) @ Wm1^T + bm1) @ Wm2^T + bm2.

Host does sharding, index-only layout transforms (including the node-row
pre-gather by edge endpoints between the NEFFs), and the final top-k /
relabel index assembly on device-computed scores.
"""

import numpy as np

G, NPG, EPG = 128, 256, 4096
N, E = G * NPG, G * EPG
IN = H = 64
K = EPG // 2
NCORES = 8
GPC = G // NCORES
NPS = GPC * NPG            # 4096 nodes / shard
EPS = GPC * EPG            # 65536 edges / shard

_cache = {}


def _patch_tile():
    import concourse.mybir as mybir
    import concourse.tile as tile_mod
    from concourse.vector_clock import ScopedClock

    if getattr(tile_mod.TileContext, "_ant_patched", False):
        return

    def _drain_and_barrier(self, tick_clock, wait_clock):
        drain_binst = self.nc.sync.drain()
        wait_clock.add_sem_waits(
            drain_binst.ins, ScopedClock({None: tick_clock.global_clock})
        )
        mi = drain_binst.ins
        si = mi.sync_info
        if si is not None and si.on_wait is not None and len(si.on_wait) > 1:
            waits = list(si.on_wait)
            si.on_wait = [waits[0]]
            for w in waits[1:]:
                d2 = self.nc.sync.drain().ins
                if d2.sync_info is None:
                    d2.sync_info = mybir.SyncInfo(on_wait=[w], on_update=[])
                else:
                    d2.sync_info.on_wait = [w]
        self.nc.all_engine_barrier()
        popped = self.nc._tile_sem_poison_stack.pop()
        assert popped is self._sem_poison
        self.nc.clear_and_free_semaphores(list(self.sems.allocated().values()))
        self.nc.all_engine_barrier()

    tile_mod.TileContext._drain_and_barrier = _drain_and_barrier
    tile_mod.TileContext._ant_patched = True


def _split_sync_waits(nc):
    """walrus here allows one sync-wait per instruction; hoist extras onto
    injected same-engine drains."""
    import concourse.mybir as mybir

    for f in nc.m.functions:
        for bb in f.blocks:
            out = []
            changed = False
            for inst in bb.instructions:
                si = inst.sync_info
                waits = list(si.on_wait) if (si is not None and si.on_wait) else []
                if len(waits) > 1:
                    changed = True
                    for w in waits[:-1]:
                        d = mybir.InstDrain(name=f"WF-{nc.next_id()}", ins=[], outs=[])
                        d.engine = inst.engine
                        d.sync_info = mybir.SyncInfo(on_wait=[w], on_update=[])
                        out.append(d)
                    si.on_wait = [waits[-1]]
                out.append(inst)
            if changed:
                bb.instructions = out


def _build_neff_a():
    import concourse.bass as bass
    import concourse.mybir as mybir
    from concourse.tile import TileContext
    from concourse.masks import make_identity

    _patch_tile()
    f32 = mybir.dt.float32
    nc = bass.Bass()
    xT_e = nc.declare_dram_parameter("xT", [64, NPS], f32, isOutput=False)
    # wg layout host-prepped: [128, GPC*512]; wg[p, g*512 + kp*256 + d]
    #   = W_g[g, s = kp*128 + p, d]  (s on partitions, split in 2 k-passes)
    wg_e = nc.declare_dram_parameter("wg", [128, GPC * 512], f32, isOutput=False)
    w1_e = nc.declare_dram_parameter("w1", [64, 192], f32, isOutput=False)
    w2_e = nc.declare_dram_parameter("w2", [64, 192], f32, isOutput=False)
    b1_e = nc.declare_dram_parameter("b1", [128, 192], f32, isOutput=False)
    b2_e = nc.declare_dram_parameter("b2", [128, 192], f32, isOutput=False)
    h2_e = nc.declare_dram_parameter("h2", [NPS, 64], f32, isOutput=True)

    with TileContext(nc) as tc:
        with (
            tc.tile_pool(name="const", bufs=1) as cpool,
            tc.tile_pool(name="abcp", bufs=1) as apool,
            tc.tile_pool(name="work", bufs=4) as wpool,
            tc.tile_pool(name="ps_abc", bufs=2, space="PSUM") as p_abc,
            tc.tile_pool(name="ps_agg", bufs=2, space="PSUM") as p_agg,
            tc.tile_pool(name="ps_tp", bufs=2, space="PSUM") as p_tp,
        ):
            xT = cpool.tile([64, NPS], f32)
            nc.sync.dma_start(out=xT[:], in_=xT_e[:])
            wg = cpool.tile([128, GPC * 512], f32)
            nc.sync.dma_start(out=wg[:], in_=wg_e[:])
            w1 = cpool.tile([64, 192], f32)
            nc.sync.dma_start(out=w1[:], in_=w1_e[:])
            w2 = cpool.tile([64, 192], f32)
            nc.sync.dma_start(out=w2[:], in_=w2_e[:])
            b1 = cpool.tile([128, 192], f32)
            nc.sync.dma_start(out=b1[:], in_=b1_e[:])
            b2 = cpool.tile([128, 192], f32)
            nc.sync.dma_start(out=b2[:], in_=b2_e[:])
            ident = cpool.tile([128, 128], f32)
            make_identity(nc, ident[:])
            ones = cpool.tile([128, 1], f32)
            nc.vector.memset(ones[:], 1.0)

            def layer(src_fT, wcat, brep, relu, out_cb, tagp):
                # 1) ABC[c] = node-major [128, 192] (A|B|C) for 32 node-chunks
                abc_tiles = []
                for c in range(32):
                    ps = p_abc.tile([128, 192], f32, tag="abc_ps")
                    nc.tensor.matmul(ps[:], src_fT(c), wcat[:],
                                     start=True, stop=True)
                    abc = apool.tile([128, 192], f32, tag=f"{tagp}abc{c}")
                    nc.vector.tensor_add(abc[:], ps[:], brep[:])
                    abc_tiles.append(abc)
                # 2) per (graph, d-half): agg (+ deg via ones column)
                for g in range(GPC):
                    for hh in range(2):
                        ps = p_agg.tile([128, 65], f32, tag="agg_ps")
                        for kp in range(2):
                            lhsT = wg[:, g * 512 + kp * 256 + hh * 128:
                                      g * 512 + kp * 256 + hh * 128 + 128]
                            rhs = abc_tiles[g * 2 + kp]
                            nc.tensor.matmul(ps[:, 0:64], lhsT, rhs[:, 0:64],
                                             start=(kp == 0), stop=(kp == 1))
                        for kp in range(2):
                            lhsT = wg[:, g * 512 + kp * 256 + hh * 128:
                                      g * 512 + kp * 256 + hh * 128 + 128]
                            nc.tensor.matmul(ps[:, 64:65], lhsT, ones[:],
                                             start=(kp == 0), stop=(kp == 1))
                        c = g * 2 + hh
                        abc = abc_tiles[c]
                        t1 = wpool.tile([128, 64], f32, tag="t1")
                        nc.vector.tensor_scalar_mul(t1[:], abc[:, 64:128],
                                                    ps[:, 64:65])
                        t2 = wpool.tile([128, 64], f32, tag="t2")
                        nc.vector.tensor_sub(t2[:], ps[:, 0:64], t1[:])
                        hout = wpool.tile([128, 64], f32, tag="hout")
                        nc.vector.tensor_add(hout[:], t2[:], abc[:, 128:192])
                        if relu:
                            nc.scalar.activation(
                                hout[:], hout[:],
                                mybir.ActivationFunctionType.Relu)
                        out_cb(c, hout)

            h1T_tiles = [None] * 32

            def l1_out(c, hout):
                tp = p_tp.tile([64, 128], f32, tag="h1t_ps")
                nc.tensor.transpose(tp[:], hout[:], ident[:])
                h1T = apool.tile([64, 128], f32, tag=f"h1T{c}")
                nc.vector.tensor_copy(h1T[:], tp[:])
                h1T_tiles[c] = h1T

            layer(lambda c: xT[:, c * 128:(c + 1) * 128], w1, b1, True,
                  l1_out, "L1")

            def l2_out(c, hout):
                nc.sync.dma_start(out=h2_e[c * 128:(c + 1) * 128, :], in_=hout[:])

            layer(lambda c: h1T_tiles[c][:], w2, b2, False, l2_out, "L2")

    _split_sync_waits(nc)
    return nc


def _build_neff_b():
    """Edge scoring with ON-DEVICE gather: per 128-edge block, two
    indirect DMAs fetch h2[row]/h2[col] rows, a PE transpose makes them
    feature-major, then the 512-wide z/relu/score pipeline runs."""
    import concourse.bass as bass
    import concourse.mybir as mybir
    from concourse.tile import TileContext
    from concourse.masks import make_identity

    _patch_tile()
    f32 = mybir.dt.float32
    i32 = mybir.dt.int32
    nc = bass.Bass()
    h2_e = nc.declare_dram_parameter("h2s", [NPS, 64], f32, isOutput=False)
    ri_e = nc.declare_dram_parameter("ridx", [128, EPS // 128], i32,
                                     isOutput=False)
    ci_e = nc.declare_dram_parameter("cidx", [128, EPS // 128], i32,
                                     isOutput=False)
    wm1_e = nc.declare_dram_parameter("wm1", [128, 256], f32, isOutput=False)
    bm1_e = nc.declare_dram_parameter("bm1h", [128, 2], f32, isOutput=False)
    wm2_e = nc.declare_dram_parameter("wm2", [128, 2], f32, isOutput=False)
    bm2_e = nc.declare_dram_parameter("bm2v", [1, 1], f32, isOutput=False)
    sc_e = nc.declare_dram_parameter("sc", [1, EPS], f32, isOutput=True)

    CH = 512
    NB = CH // 128
    with TileContext(nc) as tc:
        with (
            tc.tile_pool(name="const", bufs=1) as cpool,
            tc.tile_pool(name="work", bufs=4) as wpool,
            tc.tile_pool(name="gath", bufs=2 * NB) as gpool,
            tc.tile_pool(name="ps_t", bufs=2, space="PSUM") as p_t,
            tc.tile_pool(name="ps_z", bufs=2, space="PSUM") as p_z,
            tc.tile_pool(name="ps_s", bufs=2, space="PSUM") as p_s,
        ):
            ridx = cpool.tile([128, EPS // 128], i32)
            nc.sync.dma_start(out=ridx[:], in_=ri_e[:])
            cidx = cpool.tile([128, EPS // 128], i32)
            nc.sync.dma_start(out=cidx[:], in_=ci_e[:])
            wm1 = cpool.tile([128, 256], f32)
            nc.sync.dma_start(out=wm1[:], in_=wm1_e[:])
            bm1h = cpool.tile([128, 2], f32)
            nc.sync.dma_start(out=bm1h[:], in_=bm1_e[:])
            wm2 = cpool.tile([128, 2], f32)
            nc.sync.dma_start(out=wm2[:], in_=wm2_e[:])
            bm2 = cpool.tile([1, 1], f32)
            nc.sync.dma_start(out=bm2[:], in_=bm2_e[:])
            ident = cpool.tile([128, 128], f32)
            make_identity(nc, ident[:])

            for c0 in range(0, EPS, CH):
                erT = wpool.tile([128, CH], f32, tag="erT")
                for b in range(NB):
                    blk = c0 // 128 + b
                    er2 = gpool.tile([128, 128], f32, tag="er2")
                    nc.gpsimd.indirect_dma_start(
                        out=er2[:, 0:64], out_offset=None, in_=h2_e[:],
                        in_offset=bass.IndirectOffsetOnAxis(
                            ap=ridx[:, blk:blk + 1], axis=0))
                    nc.gpsimd.indirect_dma_start(
                        out=er2[:, 64:128], out_offset=None, in_=h2_e[:],
                        in_offset=bass.IndirectOffsetOnAxis(
                            ap=cidx[:, blk:blk + 1], axis=0))
                    tp = p_t.tile([128, 128], f32, tag="tp")
                    nc.tensor.transpose(tp[:], er2[:], ident[:])
                    nc.vector.tensor_copy(erT[:, b * 128:(b + 1) * 128], tp[:])
                sps = p_s.tile([1, CH], f32, tag="sps")
                for hh in range(2):
                    zps = p_z.tile([128, CH], f32, tag="zps")
                    nc.tensor.matmul(zps[:], wm1[:, hh * 128:(hh + 1) * 128],
                                     erT[:], start=True, stop=True)
                    y = wpool.tile([128, CH], f32, tag="y")
                    nc.scalar.activation(y[:], zps[:],
                                         mybir.ActivationFunctionType.Relu,
                                         bias=bm1h[:, hh:hh + 1])
                    nc.tensor.matmul(sps[:], wm2[:, hh:hh + 1], y[:],
                                     start=(hh == 0), stop=(hh == 1))
                sc = wpool.tile([1, CH], f32, tag="sc")
                nc.vector.tensor_scalar_add(sc[:], sps[:], bm2[:])
                nc.sync.dma_start(out=sc_e[:, c0:c0 + CH], in_=sc[:])

    _split_sync_waits(nc)
    return nc


def _get_neffs():
    if "a" not in _cache:
        _cache["a"] = _build_neff_a()
        _cache["b"] = _build_neff_b()
    return _cache["a"], _cache["b"]


last_device_wall_s = None


def kernel(x, edge_attr, W11, b11, W12, W13, b13, W21, b21, W22, W23, b23,
           Wm1, bm1, Wm2, bm2, edge_index, batch):
    import time as _time

    from concourse.bass_utils import run_bass_kernel_spmd

    global last_device_wall_s

    x = np.asarray(x, np.float32)
    edge_attr = np.asarray(edge_attr, np.float32)
    ei = np.asarray(edge_index)
    idt = ei.dtype
    nc_a, nc_b = _get_neffs()

    # ---- host prep: format conversions (index-driven layout) ----
    wg = np.zeros((G, NPG, NPG), np.float32)
    np.add.at(wg, (ei[0] // NPG, ei[0] % NPG, ei[1] % NPG),
              edge_attr.astype(np.float32))
    wcat1 = np.ascontiguousarray(
        np.concatenate([W11.T, W12.T, W13.T], axis=1), np.float32)
    wcat2 = np.ascontiguousarray(
        np.concatenate([W21.T, W22.T, W23.T], axis=1), np.float32)
    z64 = np.zeros(64, np.float32)
    b1rep = np.tile(np.concatenate([b11, z64, b13])[None, :], (128, 1)).astype(
        np.float32)
    b2rep = np.tile(np.concatenate([b21, z64, b23])[None, :], (128, 1)).astype(
        np.float32)

    in_maps_a = []
    for c in range(NCORES):
        xs = x[c * NPS:(c + 1) * NPS]
        # wg device layout [128, GPC*512]: [p, g*512 + kp*256 + d]
        wgs = wg[c * GPC:(c + 1) * GPC]                # [GPC, 256, 256]
        wgl = np.ascontiguousarray(
            wgs.reshape(GPC, 2, 128, 256).transpose(2, 0, 1, 3).reshape(
                128, GPC * 512))
        in_maps_a.append({
            "xT": np.ascontiguousarray(xs.T),
            "wg": wgl,
            "w1": wcat1, "w2": wcat2, "b1": b1rep, "b2": b2rep,
        })
    _t0 = _time.time()
    res_a = run_bass_kernel_spmd(nc_a, in_maps_a, list(range(NCORES)))
    _dev_a = _time.time() - _t0

    # ---- phase B: shard-local node tables + int32 edge-endpoint indices ----
    wm1T = np.ascontiguousarray(np.asarray(Wm1, np.float32).T)          # [128,256]
    bm1h = np.ascontiguousarray(
        np.asarray(bm1, np.float32).reshape(2, 128).T)                   # [128,2]
    wm2h = np.ascontiguousarray(
        np.asarray(Wm2, np.float32).reshape(2, 128).T)                   # [128,2]
    bm2v = np.full((1, 1), np.asarray(bm2).reshape(-1)[0], np.float32)
    in_maps_b = []
    for c in range(NCORES):
        e0 = c * EPS
        # edge e = blk*128 + p; shard-local node ids
        r = (ei[0, e0:e0 + EPS] - c * NPS).astype(np.int32)
        cl = (ei[1, e0:e0 + EPS] - c * NPS).astype(np.int32)
        in_maps_b.append({
            "h2s": res_a.results[c]["h2"],
            "ridx": np.ascontiguousarray(r.reshape(-1, 128).T),
            "cidx": np.ascontiguousarray(cl.reshape(-1, 128).T),
            "wm1": wm1T, "bm1h": bm1h, "wm2": wm2h, "bm2v": bm2v})
    _t0 = _time.time()
    res_b = run_bass_kernel_spmd(nc_b, in_maps_b, list(range(NCORES)))
    _dev_b = _time.time() - _t0
    last_device_wall_s = _dev_a + _dev_b
    score = np.concatenate(
        [res_b.results[c]["sc"].reshape(-1) for c in range(NCORES)])

    # ---- host index assembly (mirrors reference) ----
    s = score.reshape(G, EPG)
    order = np.argsort(-s, axis=1, kind="stable")
    keep, drop = order[:, :K], order[:, K:]
    causal_w = np.take_along_axis(s, keep, axis=1).reshape(-1)
    conf_w = -np.take_along_axis(s, drop, axis=1).reshape(-1)
    eoffs = (np.arange(G) * EPG)[:, None]
    kg = (keep + eoffs).reshape(-1)
    dg = (drop + eoffs).reshape(-1)
    causal_ei = ei[:, kg]
    conf_ei = ei[:, dg]

    def relabel(e):
        present = np.zeros((N,), np.int32)
        present[e.reshape(-1)] = 1
        nid = np.where(present == 1, np.cumsum(present) - 1, -1).astype(np.int32)
        return nid[e].astype(idt), present.astype(idt)

    causal_rel, causal_mask = relabel(causal_ei)
    conf_rel, conf_mask = relabel(conf_ei)
    out = np.concatenate([score, causal_w, conf_w]).astype(np.float32)
    return out, causal_rel, conf_rel, causal_mask, conf_mask
